# revision 28
# baseline (speedup 1.0000x reference)
"""ChebConv (K=5) Trainium2 kernel, 8 NeuronCores.

Strategy (node sharding):
  - Nodes are sharded across 8 cores (5120 rows/core, padded N=40960).
  - Each SpMM stage: per-core dma_gather of source rows by edge col index,
    then TensorE matmuls with host-precomputed lap-weighted indicator
    matrices M[e, n] = 2*lap(e) accumulate segment sums in PSUM.
  - Chebyshev recursion T_k = 2 L T_{k-1} - T_{k-2} is realized as
    PSUM accumulation: sum_t M2[t].T @ G[t] + (-I).T @ T_{k-2}.
  - After each stage an AllGather rebuilds the full node table for the
    next stage's gathers.
  - GEMM out += T_k @ W_k is fused per node tile: T_k tiles are
    re-loaded transposed via DMA-transpose and fed to TensorE.
  - Everything bf16 on the wire / fp32 in PSUM.

Host-side prep computes degrees, Laplacian edge values, the slot
permutation (slots sorted by edge count so the shared SPMD profile is
tight), lo/hi gather split (int16 index limit), and all index/indicator
tensors.
"""

import os
import numpy as np
import ml_dtypes

bf16 = ml_dtypes.bfloat16

CFG_FULL = dict(
    N=40000, C=256, K=5, NC=8, TILE=128,
    NPAD=40960, SHARD=5120, HALF=32768, TBMAX=30, CH=10,
)

LAST_EXEC_NS = None
_PROGRAM_CACHE = {}


# --------------------------------------------------------------------------
# host preprocessing
# --------------------------------------------------------------------------

def _prep(x, edge_index, edge_weight, cfg):
    N, C, K = cfg["N"], cfg["C"], cfg["K"]
    NC, TILE = cfg["NC"], cfg["TILE"]
    NPAD, SHARD, HALF = cfg["NPAD"], cfg["SHARD"], cfg["HALF"]
    NT = SHARD // TILE
    NWIN = NPAD // TILE

    row = np.asarray(edge_index[0], dtype=np.int64)
    col = np.asarray(edge_index[1], dtype=np.int64)
    ew = np.asarray(edge_weight, dtype=np.float32)
    keep = row != col
    deg = np.bincount(row[keep], minlength=N).astype(np.float32)
    dis = np.where(deg > 0, 1.0 / np.sqrt(np.maximum(deg, 1.0)), 0.0).astype(np.float32)
    lap = (-dis[row] * np.where(keep, ew, 0.0) * dis[col]).astype(np.float32)
    nz = lap != 0.0
    row, col, lap = row[nz], col[nz], lap[nz]
    m2val = 2.0 * lap  # stage-1 copy uses scale 0.5 to undo the factor 2

    # slot permutation: per core, windows sorted by edge count descending
    win = row // TILE
    wcnt = np.bincount(win, minlength=NWIN)
    perm = np.zeros((NC, NT), dtype=np.int64)
    for c in range(NC):
        wins = np.arange(c * NT, (c + 1) * NT)
        perm[c] = wins[np.argsort(-wcnt[wins], kind="stable")]
    slotpos = np.zeros(NWIN, dtype=np.int64)
    for c in range(NC):
        slotpos[perm[c]] = np.arange(NT)
    nodes = np.arange(N)
    # out_shard rows are core-local (slot*TILE+i): unshard with pos_out.
    pos_out = (nodes // SHARD) * SHARD + slotpos[nodes // TILE] * TILE + nodes % TILE
    # table layout is chunk-major so each per-chunk AllGather output block
    # [all cores' slots c*CH..c*CH+CH) lands contiguously.
    CH = cfg["CH"]
    slot_n = slotpos[nodes // TILE]
    pos = ((slot_n // CH) * (NC * CH * TILE) + (nodes // SHARD) * (CH * TILE)
           + (slot_n % CH) * TILE + nodes % TILE)

    tcol = pos[col]                  # table position of each edge's source
    islo = tcol < HALF
    core_of = row // SHARD
    slot_of = slotpos[win]
    nloc = row % TILE                # local dst row within its tile

    # per (core, slot) lo/hi counts -> shared profile
    keyc = core_of * NT + slot_of
    lo_cnt = np.bincount(keyc[islo], minlength=NC * NT).reshape(NC, NT)
    hi_cnt = np.bincount(keyc[~islo], minlength=NC * NT).reshape(NC, NT)
    L = np.maximum((-(-lo_cnt // TILE)).max(axis=0), 1)
    H = (-(-hi_cnt // TILE)).max(axis=0)
    assert (lo_cnt <= L[None, :] * TILE).all() and (hi_cnt <= H[None, :] * TILE).all()
    T = L + H
    NSUB = int(T.sum())

    # batches of consecutive slots with sum(T) <= TBMAX, never straddling a
    # chunk boundary (per-chunk AllGather fires after its last batch)
    TBMAX = cfg["TBMAX"]
    batches = []  # (j0, j1_excl, gstart, nlo, nhi)
    j = 0
    gstart = 0
    while j < NT:
        jcap = ((j // CH) + 1) * CH
        j1 = j
        tot = 0
        while j1 < jcap and tot + T[j1] <= TBMAX:
            tot += T[j1]
            j1 += 1
        nlo = int(L[j:j1].sum())
        nhi = int(H[j:j1].sum())
        batches.append((j, j1, gstart, nlo, nhi))
        j = j1
        gstart += nlo + nhi
    assert gstart == NSUB

    # G-column (== M subtile index) of (slot j, subtile t):
    # within a batch, lo runs of all slots first, then hi runs.
    gcol_lo = np.zeros(NT, dtype=np.int64)   # first lo column of slot j
    gcol_hi = np.zeros(NT, dtype=np.int64)
    for (j0, j1, gs, nlo, nhi) in batches:
        o = gs
        for j in range(j0, j1):
            gcol_lo[j] = o
            o += L[j]
        for j in range(j0, j1):
            gcol_hi[j] = o
            o += H[j]

    # per-core M tiles and gather indices
    m_tiles_all, idx_all = [], []
    for c in range(NC):
        m = np.zeros((NSUB, TILE, TILE), dtype=np.float32)
        idx = np.zeros(NSUB * TILE, dtype=np.int64)  # table idx per gather row
        sel_c = core_of == c
        for j in range(NT):
            sel = sel_c & (slot_of == j)
            for hi in (False, True):
                s = sel & (islo != hi)
                tc_, nl_, va_ = tcol[s], nloc[s], m2val[s]
                o = np.argsort(tc_, kind="stable")
                tc_, nl_, va_ = tc_[o], nl_[o], va_[o]
                base = gcol_hi[j] if hi else gcol_lo[j]
                nsub = H[j] if hi else L[j]
                e = np.arange(tc_.size)
                sub = base + e // TILE
                erow = e % TILE
                m[sub, erow, nl_] = va_
                gi = tc_ - (HALF if hi else 0)
                idx[base * TILE:(base + nsub) * TILE] = (HALF - HALF) if hi else 0
                idx[base * TILE + e] = gi
                # padding rows keep idx 0 (valid row, M row is zero)
        m_tiles_all.append(m.astype(bf16).reshape(NSUB * TILE, TILE))
        # wrap idx into [128, NSUB*8] int16: row i -> (partition i%16, col i//16),
        # replicated over the 8 partition groups
        iw = idx.reshape(NSUB, TILE // 16, 16).astype(np.int16)  # [sub, 8, 16]
        arr = np.zeros((128, NSUB * (TILE // 16)), dtype=np.int16)
        cols = iw.transpose(0, 1, 2).reshape(NSUB * (TILE // 16), 16)  # col-major over (sub, s)
        for rep in range(8):
            arr[rep * 16:(rep + 1) * 16, :] = cols.T
        idx_all.append(arr)

    # tables
    x = np.asarray(x, dtype=np.float32)
    x_table = np.zeros((NPAD, C), dtype=bf16)
    x_table[pos] = x.astype(bf16)

    meta = dict(L=L, H=H, T=T, NSUB=NSUB, batches=batches,
                gcol_lo=gcol_lo, gcol_hi=gcol_hi, pos=pos, pos_out=pos_out)
    return m_tiles_all, idx_all, x_table, meta


# --------------------------------------------------------------------------
# device program
# --------------------------------------------------------------------------

def _build(cfg, meta, mode="full"):
    import concourse.bacc as bacc
    import concourse.mybir as mybir
    import concourse.tile as tile
    from concourse.library_config import mlp
    do_gemm = mode in ("full", "nospmm", "noag")
    do_spmm = mode in ("full", "nogemm", "noag")

    C, K, NC, TILE = cfg["C"], cfg["K"], cfg["NC"], cfg["TILE"]
    NPAD, SHARD, HALF, TBMAX = cfg["NPAD"], cfg["SHARD"], cfg["HALF"], cfg["TBMAX"]
    NT = SHARD // TILE
    L, H, T = meta["L"], meta["H"], meta["T"]
    NSUB, batches = meta["NSUB"], meta["batches"]
    gcol_lo, gcol_hi = meta["gcol_lo"], meta["gcol_hi"]
    IDXW = TILE // 16

    nc = bacc.Bacc("TRN2", target_bir_lowering=False, debug=False,
                   num_devices=NC, num_swdge_queues=2,
                   dynamic_dma_scratch_size=32768)
    dt = mybir.dt
    x_table = nc.dram_tensor("x_table", [NPAD, C], dt.bfloat16, kind="ExternalInput")
    x_shard = nc.dram_tensor("x_shard", [SHARD, C], dt.bfloat16, kind="ExternalInput")
    m_in = nc.dram_tensor("m_tiles", [NSUB * TILE, TILE], dt.bfloat16, kind="ExternalInput")
    idx_in = nc.dram_tensor("idxs", [128, NSUB * IDXW], dt.int16, kind="ExternalInput")
    w_in = nc.dram_tensor("w_chunks", [2 * K * 128, C], dt.bfloat16, kind="ExternalInput")
    bias_in = nc.dram_tensor("bias_bcast", [128, C], dt.float32, kind="ExternalInput")
    negi_in = nc.dram_tensor("neg_id", [128, 128], dt.bfloat16, kind="ExternalInput")
    posi_in = nc.dram_tensor("pos_id", [128, 128], dt.bfloat16, kind="ExternalInput")
    out_t = nc.dram_tensor("out_shard", [SHARD, C], dt.float32, kind="ExternalOutput")
    CH = cfg["CH"]
    NCH = NT // CH
    CHROWS = CH * TILE

    with tile.TileContext(nc) as tc:
        nc.gpsimd.load_library(mlp)
        with (
            tc.tile_pool(name="const", bufs=1) as const,
            tc.tile_pool(name="acc", bufs=NT) as accp,
            tc.tile_pool(name="g", bufs=3) as gp,
            tc.tile_pool(name="m", bufs=2) as mp,
            tc.tile_pool(name="tn", bufs=3) as tnp,
            tc.tile_pool(name="tp", bufs=3) as tpp,
            tc.tile_pool(name="tt", bufs=6) as ttp,
            tc.tile_pool(name="sp", bufs=3, space="PSUM") as spp,
            tc.tile_pool(name="gp", bufs=2, space="PSUM") as gpp,
            tc.tile_pool(name="tq", bufs=2, space="PSUM") as tqp,
            tc.tile_pool(name="dram", bufs=1, space="DRAM") as dram,
        ):
            idx_sb = const.tile([128, NSUB * IDXW], dt.int16)
            nc.sync.dma_start(idx_sb[:], idx_in[:])
            w_sb = const.tile([128, 2 * K, C], dt.bfloat16)
            nc.sync.dma_start(w_sb[:], w_in[:].rearrange("(w p) n -> p w n", p=128))
            bias_sb = const.tile([128, C], dt.float32)
            nc.sync.dma_start(bias_sb[:], bias_in[:])
            negi_sb = const.tile([128, 128], dt.bfloat16)
            nc.sync.dma_start(negi_sb[:], negi_in[:])
            posi_sb = const.tile([128, 128], dt.bfloat16)
            nc.sync.dma_start(posi_sb[:], posi_in[:])

            S = {}      # gather tables per stage (stage k gathers from S[k])
            SC = {}     # per-chunk AllGather outputs (Shared; single writer
                        # each — the checker rejects multi-writer Shared DRAM)
            B = {}      # per-core shard of T_k (AG input / local reload)
            B[0] = x_shard
            S[1] = x_table
            for k in range(2, K):
                S[k] = dram.tile([NPAD, C], dt.bfloat16, name=f"s_table_{k}")
                SC[k] = [dram.tile([NC * CHROWS, C], dt.bfloat16,
                                   addr_space="Shared", name=f"sc_{k}_{c}")
                         for c in range(NCH)]
            for k in range(1, K):
                B[k] = dram.tile([SHARD, C], dt.bfloat16, name=f"b_shard_{k}")

            acc = []
            for j in range(NT):
                a = accp.tile([128, C], dt.float32, tag="acc", name=f"acc_{j}")
                acc.append(a)

            def gemm(j, k, t_sb):
                """acc[j] (+)= T_k[tile j] @ W_k (+ bias at k==0).

                t_sb: SBUF [128, C] bf16 tile holding T_k rows. Transposed
                on TensorE (lhsT=t_sb chunk, rhs=I) so no DMA-transpose is
                needed — DMA-transpose overlapping a collective hangs the
                chip, which would serialize the chunked AllGathers."""
                if not do_gemm:
                    if k == 0:
                        nc.vector.tensor_copy(acc[j][:], bias_sb[:])
                    return
                gps = gpp.tile([128, C], dt.float32, tag="gps")
                for ch in range(2):
                    pT = tqp.tile([128, 128], dt.float32, tag="pt")
                    nc.tensor.matmul(pT[:], lhsT=t_sb[:, ch * 128:(ch + 1) * 128],
                                     rhs=posi_sb[:], start=True, stop=True)
                    tT = ttp.tile([128, 128], dt.bfloat16, tag="tt")
                    nc.scalar.activation(tT[:], pT[:],
                                         mybir.ActivationFunctionType.Copy)
                    nc.tensor.matmul(gps[:], lhsT=tT[:], rhs=w_sb[:, 2 * k + ch, :],
                                     start=(ch == 0), stop=(ch == 1))
                if k == 0:
                    nc.vector.tensor_add(acc[j][:], gps[:], bias_sb[:])
                else:
                    nc.vector.tensor_add(acc[j][:], acc[j][:], gps[:])

            # stage 0: out = x @ W0 + bias (x tiles loaded CH per DMA —
            # small transfers are fixed-cost dominated)
            for j0x in range(0, NT, CH):
                xt = tpp.tile([128, CH, C], dt.bfloat16, tag="tp")
                nc.sync.dma_start(
                    xt[:], x_shard[j0x * TILE:(j0x + CH) * TILE, :]
                    .rearrange("(t p) c -> p t c", p=128))
                for j in range(j0x, j0x + CH):
                    gemm(j, 0, xt[:, j - j0x, :])

            # stages 1..K-1; AllGather fires per chunk of CH slots so the
            # collective overlaps the rest of the stage's gather/compute
            gq = [0]  # round-robin SWDGE queue cursor
            for k in range(1, K if do_spmm else 1):
                src = S[k]
                for (j0, j1, gstart, nlo, nhi) in batches:
                    nb = nlo + nhi
                    nj = j1 - j0
                    g = gp.tile([128, TBMAX, C], dt.bfloat16, tag="g")
                    m_b = mp.tile([128, TBMAX, TILE], dt.bfloat16, tag="m")
                    nc.scalar.dma_start(
                        m_b[:, 0:nb, :],
                        m_in[gstart * TILE:(gstart + nb) * TILE, :]
                        .rearrange("(s p) n -> p s n", p=128))
                    if k > 1:
                        tpb = tpp.tile([128, CH, C], dt.bfloat16, tag="tp")
                        nc.sync.dma_start(
                            tpb[:, 0:nj, :],
                            B[k - 2][j0 * TILE:j1 * TILE, :]
                            .rearrange("(t p) c -> p t c", p=128))
                    tb = tnp.tile([128, CH, C], dt.bfloat16, tag="tn")
                    # SWDGE descriptor ring holds ~1024 descriptors per queue;
                    # one gather call emits one descriptor per index, so cap
                    # calls at SUBCAP subtiles (SUBCAP*128 indices) and
                    # alternate queues so emission never waits on drain.
                    SUBCAP = 8
                    segs = ([(o, min(SUBCAP, nlo - o), src[0:HALF, :])
                             for o in range(0, nlo, SUBCAP)] +
                            [(nlo + o, min(SUBCAP, nhi - o), src[HALF:NPAD, :])
                             for o in range(0, nhi, SUBCAP)])
                    for (o, n, s_ap) in segs:
                        nc.gpsimd.dma_gather(
                            g[:, o:o + n, :], s_ap,
                            idx_sb[:, (gstart + o) * IDXW:(gstart + o + n) * IDXW],
                            n * TILE, n * TILE, C, queue_num=gq[0])
                        gq[0] = (gq[0] + 1) % 2
                    for j in range(j0, j1):
                        psum = spp.tile([128, C], dt.float32, tag="sp")
                        subs = ([gcol_lo[j] + t for t in range(L[j])] +
                                [gcol_hi[j] + t for t in range(H[j])])
                        for ti, s in enumerate(subs):
                            nc.tensor.matmul(
                                psum[:], lhsT=m_b[:, s - gstart, :], rhs=g[:, s - gstart, :],
                                start=(ti == 0),
                                stop=(k == 1 and ti == len(subs) - 1))
                        if k > 1:
                            nc.tensor.matmul(psum[:], lhsT=negi_sb[:],
                                             rhs=tpb[:, j - j0, :],
                                             start=False, stop=True)
                        nc.scalar.activation(tb[:, j - j0, :], psum[:],
                                             mybir.ActivationFunctionType.Copy,
                                             scale=(0.5 if k == 1 else 1.0))
                        gemm(j, k, tb[:, j - j0, :])
                        if k == K - 1:
                            nc.sync.dma_start(
                                out_t[j * TILE:(j + 1) * TILE, :], acc[j][:])
                    if k < K - 1:
                        nc.sync.dma_start(
                            B[k][j0 * TILE:j1 * TILE, :]
                            .rearrange("(t p) c -> p t c", p=128),
                            tb[:, 0:nj, :])
                    # fire this chunk's AllGather as soon as its last slot's
                    # shard rows are written (batches never straddle chunks),
                    # then copy the Shared output into the flat gather table
                    if k < K - 1 and mode != "noag" and j1 % CH == 0:
                        c = (j1 - 1) // CH
                        nc.gpsimd.collective_compute(
                            "AllGather", mybir.AluOpType.bypass,
                            replica_groups=[list(range(NC))],
                            ins=[B[k][c * CHROWS:(c + 1) * CHROWS, :].opt()],
                            outs=[SC[k + 1][c][:].opt()])
                        nc.scalar.dma_start(
                            S[k + 1][c * NC * CHROWS:(c + 1) * NC * CHROWS, :],
                            SC[k + 1][c][:])

            if not do_spmm:
                for j in range(NT):
                    nc.sync.dma_start(out_t[j * TILE:(j + 1) * TILE, :], acc[j][:])
    nc.compile()
    return nc


# --------------------------------------------------------------------------
# entry point
# --------------------------------------------------------------------------

def kernel(x, edge_index, edge_weight, weight, bias):
    global LAST_EXEC_NS
    from concourse.bass_utils import run_bass_kernel_spmd

    cfg = CFG_FULL
    N, C, K, NC, SHARD = cfg["N"], cfg["C"], cfg["K"], cfg["NC"], cfg["SHARD"]
    x = np.asarray(x)
    weight = np.asarray(weight, dtype=np.float32)
    bias = np.asarray(bias, dtype=np.float32)

    m_tiles_all, idx_all, x_table, meta = _prep(x, edge_index, edge_weight, cfg)

    key = (tuple(meta["L"]), tuple(meta["H"]))
    if key not in _PROGRAM_CACHE:
        _PROGRAM_CACHE[key] = _build(cfg, meta)
    nc = _PROGRAM_CACHE[key]

    w_chunks = np.zeros((2 * K * 128, C), dtype=bf16)
    for k in range(K):
        for ch in range(2):
            w_chunks[(2 * k + ch) * 128:(2 * k + ch + 1) * 128] = \
                weight[k, ch * 128:(ch + 1) * 128, :].astype(bf16)
    bias_bcast = np.broadcast_to(bias, (128, C)).astype(np.float32).copy()
    neg_id = (-np.eye(128, dtype=np.float32)).astype(bf16)
    pos_id = np.eye(128, dtype=np.float32).astype(bf16)

    # core-local-order copy of x (x_table is chunk-major, so its SHARD
    # slices no longer correspond to cores)
    pos_out = meta["pos_out"]
    x_core = np.zeros((cfg["NPAD"], C), dtype=bf16)
    x_core[pos_out] = x.astype(bf16)

    in_maps = []
    for c in range(NC):
        in_maps.append({
            "x_table": x_table,
            "x_shard": x_core[c * SHARD:(c + 1) * SHARD],
            "m_tiles": m_tiles_all[c],
            "idxs": idx_all[c],
            "w_chunks": w_chunks,
            "bias_bcast": bias_bcast,
            "neg_id": neg_id,
            "pos_id": pos_id,
        })

    trace = bool(os.environ.get("CHEB_TRACE"))
    kw = {}
    if trace:
        kw = dict(trace=True, tmpdir=os.environ.get("CHEB_TRACE_DIR") or None)
    res = run_bass_kernel_spmd(nc, in_maps, core_ids=list(range(NC)), **kw)
    LAST_EXEC_NS = res.exec_time_ns

    shards = [res.results[c]["out_shard"] for c in range(NC)]
    full = np.concatenate(shards, axis=0)      # [NPAD(core-local order), C]
    out = full[pos_out]                        # back to node order
    return np.ascontiguousarray(out.astype(np.float32))



# revision 42
# speedup vs baseline: 1.2223x; 1.2223x over previous
"""ChebConv (K=5) Trainium2 kernel, 8 NeuronCores.

Strategy (node sharding):
  - Nodes are sharded across 8 cores (5120 rows/core, padded N=40960).
  - Each SpMM stage: per-core dma_gather of source rows by edge col index,
    then TensorE matmuls with host-precomputed lap-weighted indicator
    matrices M[e, n] = 2*lap(e) accumulate segment sums in PSUM.
  - Chebyshev recursion T_k = 2 L T_{k-1} - T_{k-2} is realized as
    PSUM accumulation: sum_t M2[t].T @ G[t] + (-I).T @ T_{k-2}.
  - After each stage an AllGather rebuilds the full node table for the
    next stage's gathers.
  - GEMM out += T_k @ W_k is fused per node tile: T_k tiles are
    re-loaded transposed via DMA-transpose and fed to TensorE.
  - Everything bf16 on the wire / fp32 in PSUM.

Host-side prep computes degrees, Laplacian edge values, the slot
permutation (slots sorted by edge count so the shared SPMD profile is
tight), lo/hi gather split (int16 index limit), and all index/indicator
tensors.
"""

import os
import numpy as np
import ml_dtypes

bf16 = ml_dtypes.bfloat16

CFG_FULL = dict(
    N=40000, C=256, K=5, NC=8, TILE=128,
    NPAD=40960, SHARD=5120, TBMAX=30, CH1=32,
)

LAST_EXEC_NS = None
_PROGRAM_CACHE = {}


# --------------------------------------------------------------------------
# host preprocessing
# --------------------------------------------------------------------------

def _prep(x, edge_index, edge_weight, cfg):
    N, C, K = cfg["N"], cfg["C"], cfg["K"]
    NC, TILE = cfg["NC"], cfg["TILE"]
    NPAD, SHARD = cfg["NPAD"], cfg["SHARD"]
    NT = SHARD // TILE
    NWIN = NPAD // TILE
    # lo/hi gather split = source-chunk split: chunk 0 is CH1=32 slots per
    # core (32768 table rows — the int16-friendly 80/20 split), chunk 1 the
    # remaining 8. Each seg then depends on a single per-chunk AllGather.
    CH1 = cfg["CH1"]
    HALF = NC * CH1 * TILE

    row = np.asarray(edge_index[0], dtype=np.int64)
    col = np.asarray(edge_index[1], dtype=np.int64)
    ew = np.asarray(edge_weight, dtype=np.float32)
    keep = row != col
    deg = np.bincount(row[keep], minlength=N).astype(np.float32)
    dis = np.where(deg > 0, 1.0 / np.sqrt(np.maximum(deg, 1.0)), 0.0).astype(np.float32)
    lap = (-dis[row] * np.where(keep, ew, 0.0) * dis[col]).astype(np.float32)
    nz = lap != 0.0
    row, col, lap = row[nz], col[nz], lap[nz]
    m2val = 2.0 * lap  # stage-1 copy uses scale 0.5 to undo the factor 2

    # slot permutation: per core, windows sorted by edge count ASCENDING so
    # chunk 0 (lightest slots) completes — and its AllGather fires — early,
    # while the shared SPMD profile stays tight (heavy aligns with heavy).
    win = row // TILE
    wcnt = np.bincount(win, minlength=NWIN)
    perm = np.zeros((NC, NT), dtype=np.int64)
    for c in range(NC):
        wins = np.arange(c * NT, (c + 1) * NT)
        perm[c] = wins[np.argsort(wcnt[wins], kind="stable")]
    slotpos = np.zeros(NWIN, dtype=np.int64)
    for c in range(NC):
        slotpos[perm[c]] = np.arange(NT)
    nodes = np.arange(N)
    # out_shard rows are core-local (slot*TILE+i): unshard with pos_out.
    pos_out = (nodes // SHARD) * SHARD + slotpos[nodes // TILE] * TILE + nodes % TILE
    # table layout is chunk-major so each per-chunk AllGather output block
    # lands contiguously: chunk 0 = all cores' slots [0,CH1), chunk 1 = rest.
    slot_n = slotpos[nodes // TILE]
    core_n = nodes // SHARD
    pos = np.where(
        slot_n < CH1,
        core_n * (CH1 * TILE) + slot_n * TILE + nodes % TILE,
        HALF + core_n * ((NT - CH1) * TILE) + (slot_n - CH1) * TILE
        + nodes % TILE)

    tcol = pos[col]                  # table position of each edge's source
    islo = tcol < HALF
    core_of = row // SHARD
    slot_of = slotpos[win]
    nloc = row % TILE                # local dst row within its tile

    # per (core, slot) lo/hi counts -> shared profile
    keyc = core_of * NT + slot_of
    lo_cnt = np.bincount(keyc[islo], minlength=NC * NT).reshape(NC, NT)
    hi_cnt = np.bincount(keyc[~islo], minlength=NC * NT).reshape(NC, NT)
    L = np.maximum((-(-lo_cnt // TILE)).max(axis=0), 1)
    H = (-(-hi_cnt // TILE)).max(axis=0)
    assert (lo_cnt <= L[None, :] * TILE).all() and (hi_cnt <= H[None, :] * TILE).all()
    T = L + H
    NSUB = int(T.sum())

    # batches of consecutive slots with sum(T) <= TBMAX, never straddling a
    # chunk boundary (per-chunk AllGather fires after its last batch)
    TBMAX = cfg["TBMAX"]
    batches = []  # (j0, j1_excl, gstart, nlo, nhi)
    j = 0
    gstart = 0
    while j < NT:
        jcap = CH1 if j < CH1 else NT
        j1 = j
        tot = 0
        while j1 < jcap and (j1 - j) < 10 and tot + T[j1] <= TBMAX:
            tot += T[j1]
            j1 += 1
        nlo = int(L[j:j1].sum())
        nhi = int(H[j:j1].sum())
        batches.append((j, j1, gstart, nlo, nhi))
        j = j1
        gstart += nlo + nhi
    assert gstart == NSUB

    # G-column (== M subtile index) of (slot j, subtile t):
    # within a batch, lo runs of all slots first, then hi runs.
    gcol_lo = np.zeros(NT, dtype=np.int64)   # first lo column of slot j
    gcol_hi = np.zeros(NT, dtype=np.int64)
    for (j0, j1, gs, nlo, nhi) in batches:
        o = gs
        for j in range(j0, j1):
            gcol_lo[j] = o
            o += L[j]
        for j in range(j0, j1):
            gcol_hi[j] = o
            o += H[j]

    # per-core M tiles and gather indices
    m_tiles_all, idx_all = [], []
    for c in range(NC):
        m = np.zeros((NSUB, TILE, TILE), dtype=np.float32)
        idx = np.zeros(NSUB * TILE, dtype=np.int64)  # table idx per gather row
        sel_c = core_of == c
        for j in range(NT):
            sel = sel_c & (slot_of == j)
            for hi in (False, True):
                s = sel & (islo != hi)
                tc_, nl_, va_ = tcol[s], nloc[s], m2val[s]
                o = np.argsort(tc_, kind="stable")
                tc_, nl_, va_ = tc_[o], nl_[o], va_[o]
                base = gcol_hi[j] if hi else gcol_lo[j]
                nsub = H[j] if hi else L[j]
                e = np.arange(tc_.size)
                sub = base + e // TILE
                erow = e % TILE
                m[sub, erow, nl_] = va_
                gi = tc_ - (HALF if hi else 0)
                idx[base * TILE:(base + nsub) * TILE] = (HALF - HALF) if hi else 0
                idx[base * TILE + e] = gi
                # padding rows keep idx 0 (valid row, M row is zero)
        m_tiles_all.append(m.astype(bf16).reshape(NSUB * TILE, TILE))
        # wrap idx into [128, NSUB*8] int16: row i -> (partition i%16, col i//16),
        # replicated over the 8 partition groups
        iw = idx.reshape(NSUB, TILE // 16, 16).astype(np.int16)  # [sub, 8, 16]
        arr = np.zeros((128, NSUB * (TILE // 16)), dtype=np.int16)
        cols = iw.transpose(0, 1, 2).reshape(NSUB * (TILE // 16), 16)  # col-major over (sub, s)
        for rep in range(8):
            arr[rep * 16:(rep + 1) * 16, :] = cols.T
        idx_all.append(arr)

    # tables
    x = np.asarray(x, dtype=np.float32)
    x_table = np.zeros((NPAD, C), dtype=bf16)
    x_table[pos] = x.astype(bf16)

    meta = dict(L=L, H=H, T=T, NSUB=NSUB, batches=batches,
                gcol_lo=gcol_lo, gcol_hi=gcol_hi, pos=pos, pos_out=pos_out)
    return m_tiles_all, idx_all, x_table, meta


# --------------------------------------------------------------------------
# device program
# --------------------------------------------------------------------------

def _build(cfg, meta, mode="full"):
    import concourse.bacc as bacc
    import concourse.mybir as mybir
    import concourse.tile as tile
    from concourse.library_config import mlp
    do_gemm = mode in ("full", "nospmm", "noag")
    do_spmm = mode in ("full", "nogemm", "noag")

    C, K, NC, TILE = cfg["C"], cfg["K"], cfg["NC"], cfg["TILE"]
    NPAD, SHARD, TBMAX = cfg["NPAD"], cfg["SHARD"], cfg["TBMAX"]
    CH1 = cfg["CH1"]
    HALF = NC * CH1 * TILE
    NT = SHARD // TILE
    L, H, T = meta["L"], meta["H"], meta["T"]
    NSUB, batches = meta["NSUB"], meta["batches"]
    gcol_lo, gcol_hi = meta["gcol_lo"], meta["gcol_hi"]
    IDXW = TILE // 16

    nc = bacc.Bacc("TRN2", target_bir_lowering=False, debug=False,
                   num_devices=NC, num_swdge_queues=2,
                   dynamic_dma_scratch_size=32768)
    dt = mybir.dt
    x_table = nc.dram_tensor("x_table", [NPAD, C], dt.bfloat16, kind="ExternalInput")
    x_shard = nc.dram_tensor("x_shard", [SHARD, C], dt.bfloat16, kind="ExternalInput")
    m_in = nc.dram_tensor("m_tiles", [NSUB * TILE, TILE], dt.bfloat16, kind="ExternalInput")
    idx_in = nc.dram_tensor("idxs", [128, NSUB * IDXW], dt.int16, kind="ExternalInput")
    w_in = nc.dram_tensor("w_chunks", [2 * K * 128, C], dt.bfloat16, kind="ExternalInput")
    bias_in = nc.dram_tensor("bias_bcast", [128, C], dt.float32, kind="ExternalInput")
    negi_in = nc.dram_tensor("neg_id", [128, 128], dt.bfloat16, kind="ExternalInput")
    posi_in = nc.dram_tensor("pos_id", [128, 128], dt.bfloat16, kind="ExternalInput")
    out_t = nc.dram_tensor("out_shard", [SHARD, C], dt.float32, kind="ExternalOutput")
    R0 = CH1 * TILE               # chunk-0 rows per core (4096)
    CH = 10                       # per-batch staging cap (slots per batch)

    with tile.TileContext(nc) as tc:
        nc.gpsimd.load_library(mlp)
        with (
            tc.tile_pool(name="const", bufs=1) as const,
            tc.tile_pool(name="acc", bufs=NT) as accp,
            tc.tile_pool(name="g", bufs=3) as gp,
            tc.tile_pool(name="m", bufs=2) as mp,
            tc.tile_pool(name="tn", bufs=3) as tnp,
            tc.tile_pool(name="tp", bufs=3) as tpp,
            tc.tile_pool(name="tt", bufs=6) as ttp,
            tc.tile_pool(name="sp", bufs=3, space="PSUM") as spp,
            tc.tile_pool(name="gp", bufs=2, space="PSUM") as gpp,
            tc.tile_pool(name="tq", bufs=2, space="PSUM") as tqp,
            tc.tile_pool(name="dram", bufs=1, space="DRAM") as dram,
        ):
            idx_sb = const.tile([128, NSUB * IDXW], dt.int16)
            nc.sync.dma_start(idx_sb[:], idx_in[:])
            w_sb = const.tile([128, 2 * K, C], dt.bfloat16)
            nc.sync.dma_start(w_sb[:], w_in[:].rearrange("(w p) n -> p w n", p=128))
            bias_sb = const.tile([128, C], dt.float32)
            nc.sync.dma_start(bias_sb[:], bias_in[:])
            negi_sb = const.tile([128, 128], dt.bfloat16)
            nc.sync.dma_start(negi_sb[:], negi_in[:])
            posi_sb = const.tile([128, 128], dt.bfloat16)
            nc.sync.dma_start(posi_sb[:], posi_in[:])

            SRC = {}    # per-stage (chunk0, chunk1) gather source tensors
            B = {}      # per-core shard of T_k (AG input / local reload)
            B[0] = x_shard
            SRC[1] = (x_table[0:HALF, :], x_table[HALF:NPAD, :])
            for k in range(2, K):
                # per-chunk AllGather outputs (Shared; single writer each —
                # the checker rejects multi-writer Shared DRAM)
                SRC[k] = (dram.tile([NC * R0, C], dt.bfloat16,
                                    addr_space="Shared", name=f"sc_{k}_0")[:],
                          dram.tile([NC * (SHARD - R0), C], dt.bfloat16,
                                    addr_space="Shared", name=f"sc_{k}_1")[:])
            for k in range(1, K):
                B[k] = dram.tile([SHARD, C], dt.bfloat16, name=f"b_shard_{k}")

            acc = []
            for j in range(NT):
                a = accp.tile([128, C], dt.float32, tag="acc", name=f"acc_{j}")
                acc.append(a)

            def gemm(j, k, t_sb):
                """acc[j] (+)= T_k[tile j] @ W_k (+ bias at k==0).

                t_sb: SBUF [128, C] bf16 tile holding T_k rows. Transposed
                on TensorE (lhsT=t_sb chunk, rhs=I) so no DMA-transpose is
                needed — DMA-transpose overlapping a collective hangs the
                chip, which would serialize the chunked AllGathers."""
                if not do_gemm:
                    if k == 0:
                        nc.vector.tensor_copy(acc[j][:], bias_sb[:])
                    return
                gps = gpp.tile([128, C], dt.float32, tag="gps")
                for ch in range(2):
                    pT = tqp.tile([128, 128], dt.float32, tag="pt")
                    nc.tensor.matmul(pT[:], lhsT=t_sb[:, ch * 128:(ch + 1) * 128],
                                     rhs=posi_sb[:], start=True, stop=True)
                    tT = ttp.tile([128, 128], dt.bfloat16, tag="tt")
                    nc.scalar.activation(tT[:], pT[:],
                                         mybir.ActivationFunctionType.Copy)
                    nc.tensor.matmul(gps[:], lhsT=tT[:], rhs=w_sb[:, 2 * k + ch, :],
                                     start=(ch == 0), stop=(ch == 1))
                if k == 0:
                    nc.vector.tensor_add(acc[j][:], gps[:], bias_sb[:])
                else:
                    nc.vector.tensor_add(acc[j][:], acc[j][:], gps[:])

            # stage 0: out = x @ W0 + bias (x tiles loaded CH per DMA —
            # small transfers are fixed-cost dominated)
            for j0x in range(0, NT, CH):
                xt = tpp.tile([128, CH, C], dt.bfloat16, tag="tp")
                nc.sync.dma_start(
                    xt[:], x_shard[j0x * TILE:(j0x + CH) * TILE, :]
                    .rearrange("(t p) c -> p t c", p=128))
                for j in range(j0x, j0x + CH):
                    gemm(j, 0, xt[:, j - j0x, :])

            # stages 1..K-1; AllGathers fire per chunk (slots [0,CH1) then
            # the rest) so the collectives overlap the stage's own compute
            # and the next stage's chunk-0 gathers depend only on AG_0
            gq = [0]  # round-robin SWDGE queue cursor
            for k in range(1, K if do_spmm else 1):
                src_a, src_b = SRC[k]
                for (j0, j1, gstart, nlo, nhi) in batches:
                    nb = nlo + nhi
                    nj = j1 - j0
                    g = gp.tile([128, TBMAX, C], dt.bfloat16, tag="g")
                    m_b = mp.tile([128, TBMAX, TILE], dt.bfloat16, tag="m")
                    nc.scalar.dma_start(
                        m_b[:, 0:nb, :],
                        m_in[gstart * TILE:(gstart + nb) * TILE, :]
                        .rearrange("(s p) n -> p s n", p=128))
                    if k > 1:
                        tpb = tpp.tile([128, CH, C], dt.bfloat16, tag="tp")
                        nc.sync.dma_start(
                            tpb[:, 0:nj, :],
                            B[k - 2][j0 * TILE:j1 * TILE, :]
                            .rearrange("(t p) c -> p t c", p=128))
                    tb = tnp.tile([128, CH, C], dt.bfloat16, tag="tn")
                    # SWDGE descriptor ring holds ~1024 descriptors per queue;
                    # one gather call emits one descriptor per index, so cap
                    # calls at SUBCAP subtiles (SUBCAP*128 indices) and
                    # alternate queues so emission never waits on drain.
                    SUBCAP = 8
                    segs = ([(o, min(SUBCAP, nlo - o), src_a)
                             for o in range(0, nlo, SUBCAP)] +
                            [(nlo + o, min(SUBCAP, nhi - o), src_b)
                             for o in range(0, nhi, SUBCAP)])
                    for (o, n, s_ap) in segs:
                        nc.gpsimd.dma_gather(
                            g[:, o:o + n, :], s_ap,
                            idx_sb[:, (gstart + o) * IDXW:(gstart + o + n) * IDXW],
                            n * TILE, n * TILE, C, queue_num=gq[0])
                        gq[0] = (gq[0] + 1) % 2
                    for j in range(j0, j1):
                        psum = spp.tile([128, C], dt.float32, tag="sp")
                        subs = ([gcol_lo[j] + t for t in range(L[j])] +
                                [gcol_hi[j] + t for t in range(H[j])])
                        for ti, s in enumerate(subs):
                            nc.tensor.matmul(
                                psum[:], lhsT=m_b[:, s - gstart, :], rhs=g[:, s - gstart, :],
                                start=(ti == 0),
                                stop=(k == 1 and ti == len(subs) - 1))
                        if k > 1:
                            nc.tensor.matmul(psum[:], lhsT=negi_sb[:],
                                             rhs=tpb[:, j - j0, :],
                                             start=False, stop=True)
                        nc.scalar.activation(tb[:, j - j0, :], psum[:],
                                             mybir.ActivationFunctionType.Copy,
                                             scale=(0.5 if k == 1 else 1.0))
                        gemm(j, k, tb[:, j - j0, :])
                        if k == K - 1:
                            nc.sync.dma_start(
                                out_t[j * TILE:(j + 1) * TILE, :], acc[j][:])
                    if k < K - 1:
                        nc.sync.dma_start(
                            B[k][j0 * TILE:j1 * TILE, :]
                            .rearrange("(t p) c -> p t c", p=128),
                            tb[:, 0:nj, :])
                    # fire each chunk's AllGather as soon as its last slot's
                    # shard rows are written (batches never straddle chunks)
                    if k < K - 1 and mode != "noag" and j1 in (CH1, NT):
                        if j1 == CH1:
                            nc.gpsimd.collective_compute(
                                "AllGather", mybir.AluOpType.bypass,
                                replica_groups=[list(range(NC))],
                                ins=[B[k][0:R0, :].opt()],
                                outs=[SRC[k + 1][0].opt()])
                        else:
                            nc.gpsimd.collective_compute(
                                "AllGather", mybir.AluOpType.bypass,
                                replica_groups=[list(range(NC))],
                                ins=[B[k][R0:SHARD, :].opt()],
                                outs=[SRC[k + 1][1].opt()])

            if not do_spmm:
                for j in range(NT):
                    nc.sync.dma_start(out_t[j * TILE:(j + 1) * TILE, :], acc[j][:])
    nc.compile()
    return nc


# --------------------------------------------------------------------------
# entry point
# --------------------------------------------------------------------------

def kernel(x, edge_index, edge_weight, weight, bias):
    global LAST_EXEC_NS
    from concourse.bass_utils import run_bass_kernel_spmd

    cfg = CFG_FULL
    N, C, K, NC, SHARD = cfg["N"], cfg["C"], cfg["K"], cfg["NC"], cfg["SHARD"]
    x = np.asarray(x)
    weight = np.asarray(weight, dtype=np.float32)
    bias = np.asarray(bias, dtype=np.float32)

    m_tiles_all, idx_all, x_table, meta = _prep(x, edge_index, edge_weight, cfg)

    key = (tuple(meta["L"]), tuple(meta["H"]))
    if key not in _PROGRAM_CACHE:
        _PROGRAM_CACHE[key] = _build(cfg, meta)
    nc = _PROGRAM_CACHE[key]

    w_chunks = np.zeros((2 * K * 128, C), dtype=bf16)
    for k in range(K):
        for ch in range(2):
            w_chunks[(2 * k + ch) * 128:(2 * k + ch + 1) * 128] = \
                weight[k, ch * 128:(ch + 1) * 128, :].astype(bf16)
    bias_bcast = np.broadcast_to(bias, (128, C)).astype(np.float32).copy()
    neg_id = (-np.eye(128, dtype=np.float32)).astype(bf16)
    pos_id = np.eye(128, dtype=np.float32).astype(bf16)

    # core-local-order copy of x (x_table is chunk-major, so its SHARD
    # slices no longer correspond to cores)
    pos_out = meta["pos_out"]
    x_core = np.zeros((cfg["NPAD"], C), dtype=bf16)
    x_core[pos_out] = x.astype(bf16)

    in_maps = []
    for c in range(NC):
        in_maps.append({
            "x_table": x_table,
            "x_shard": x_core[c * SHARD:(c + 1) * SHARD],
            "m_tiles": m_tiles_all[c],
            "idxs": idx_all[c],
            "w_chunks": w_chunks,
            "bias_bcast": bias_bcast,
            "neg_id": neg_id,
            "pos_id": pos_id,
        })

    trace = bool(os.environ.get("CHEB_TRACE"))
    kw = {}
    if trace:
        kw = dict(trace=True, tmpdir=os.environ.get("CHEB_TRACE_DIR") or None)
    res = run_bass_kernel_spmd(nc, in_maps, core_ids=list(range(NC)), **kw)
    LAST_EXEC_NS = res.exec_time_ns

    shards = [res.results[c]["out_shard"] for c in range(NC)]
    full = np.concatenate(shards, axis=0)      # [NPAD(core-local order), C]
    out = full[pos_out]                        # back to node order
    return np.ascontiguousarray(out.astype(np.float32))



# revision 43
# speedup vs baseline: 1.4002x; 1.1456x over previous
"""ChebConv (K=5) Trainium2 kernel, 8 NeuronCores.

Strategy (node sharding):
  - Nodes are sharded across 8 cores (5120 rows/core, padded N=40960).
  - Each SpMM stage: per-core dma_gather of source rows by edge col index,
    then TensorE matmuls with host-precomputed lap-weighted indicator
    matrices M[e, n] = 2*lap(e) accumulate segment sums in PSUM.
  - Chebyshev recursion T_k = 2 L T_{k-1} - T_{k-2} is realized as
    PSUM accumulation: sum_t M2[t].T @ G[t] + (-I).T @ T_{k-2}.
  - After each stage an AllGather rebuilds the full node table for the
    next stage's gathers.
  - GEMM out += T_k @ W_k is fused per node tile: T_k tiles are
    re-loaded transposed via DMA-transpose and fed to TensorE.
  - Everything bf16 on the wire / fp32 in PSUM.

Host-side prep computes degrees, Laplacian edge values, the slot
permutation (slots sorted by edge count so the shared SPMD profile is
tight), lo/hi gather split (int16 index limit), and all index/indicator
tensors.
"""

import os
import numpy as np
import ml_dtypes

bf16 = ml_dtypes.bfloat16

CFG_FULL = dict(
    N=40000, C=256, K=5, NC=8, TILE=128,
    NPAD=40960, SHARD=5120, TBMAX=30, CH1=22,
)

LAST_EXEC_NS = None
_PROGRAM_CACHE = {}


# --------------------------------------------------------------------------
# host preprocessing
# --------------------------------------------------------------------------

def _prep(x, edge_index, edge_weight, cfg):
    N, C, K = cfg["N"], cfg["C"], cfg["K"]
    NC, TILE = cfg["NC"], cfg["TILE"]
    NPAD, SHARD = cfg["NPAD"], cfg["SHARD"]
    NT = SHARD // TILE
    NWIN = NPAD // TILE
    # lo/hi gather split = source-chunk split: chunk 0 is CH1=32 slots per
    # core (32768 table rows — the int16-friendly 80/20 split), chunk 1 the
    # remaining 8. Each seg then depends on a single per-chunk AllGather.
    CH1 = cfg["CH1"]
    HALF = NC * CH1 * TILE

    row = np.asarray(edge_index[0], dtype=np.int64)
    col = np.asarray(edge_index[1], dtype=np.int64)
    ew = np.asarray(edge_weight, dtype=np.float32)
    keep = row != col
    deg = np.bincount(row[keep], minlength=N).astype(np.float32)
    dis = np.where(deg > 0, 1.0 / np.sqrt(np.maximum(deg, 1.0)), 0.0).astype(np.float32)
    lap = (-dis[row] * np.where(keep, ew, 0.0) * dis[col]).astype(np.float32)
    nz = lap != 0.0
    row, col, lap = row[nz], col[nz], lap[nz]
    m2val = 2.0 * lap  # stage-1 copy uses scale 0.5 to undo the factor 2

    # slot permutation: per core, windows sorted by edge count ASCENDING so
    # chunk 0 (lightest slots) completes — and its AllGather fires — early,
    # while the shared SPMD profile stays tight (heavy aligns with heavy).
    win = row // TILE
    wcnt = np.bincount(win, minlength=NWIN)
    perm = np.zeros((NC, NT), dtype=np.int64)
    for c in range(NC):
        wins = np.arange(c * NT, (c + 1) * NT)
        perm[c] = wins[np.argsort(wcnt[wins], kind="stable")]
    slotpos = np.zeros(NWIN, dtype=np.int64)
    for c in range(NC):
        slotpos[perm[c]] = np.arange(NT)
    nodes = np.arange(N)
    # out_shard rows are core-local (slot*TILE+i): unshard with pos_out.
    pos_out = (nodes // SHARD) * SHARD + slotpos[nodes // TILE] * TILE + nodes % TILE
    # table layout is chunk-major so each per-chunk AllGather output block
    # lands contiguously: chunk 0 = all cores' slots [0,CH1), chunk 1 = rest.
    slot_n = slotpos[nodes // TILE]
    core_n = nodes // SHARD
    pos = np.where(
        slot_n < CH1,
        core_n * (CH1 * TILE) + slot_n * TILE + nodes % TILE,
        HALF + core_n * ((NT - CH1) * TILE) + (slot_n - CH1) * TILE
        + nodes % TILE)

    tcol = pos[col]                  # table position of each edge's source
    islo = tcol < HALF
    core_of = row // SHARD
    slot_of = slotpos[win]
    nloc = row % TILE                # local dst row within its tile

    # per (core, slot) lo/hi counts -> shared profile
    keyc = core_of * NT + slot_of
    lo_cnt = np.bincount(keyc[islo], minlength=NC * NT).reshape(NC, NT)
    hi_cnt = np.bincount(keyc[~islo], minlength=NC * NT).reshape(NC, NT)
    L = np.maximum((-(-lo_cnt // TILE)).max(axis=0), 1)
    H = (-(-hi_cnt // TILE)).max(axis=0)
    assert (lo_cnt <= L[None, :] * TILE).all() and (hi_cnt <= H[None, :] * TILE).all()
    T = L + H
    NSUB = int(T.sum())

    # batches of consecutive slots with sum(T) <= TBMAX, never straddling a
    # chunk boundary (per-chunk AllGather fires after its last batch)
    TBMAX = cfg["TBMAX"]
    batches = []  # (j0, j1_excl, gstart, nlo, nhi)
    j = 0
    gstart = 0
    while j < NT:
        jcap = CH1 if j < CH1 else NT
        j1 = j
        tot = 0
        while j1 < jcap and (j1 - j) < 10 and tot + T[j1] <= TBMAX:
            tot += T[j1]
            j1 += 1
        nlo = int(L[j:j1].sum())
        nhi = int(H[j:j1].sum())
        batches.append((j, j1, gstart, nlo, nhi))
        j = j1
        gstart += nlo + nhi
    assert gstart == NSUB

    # G-column (== M subtile index) of (slot j, subtile t):
    # within a batch, lo runs of all slots first, then hi runs.
    gcol_lo = np.zeros(NT, dtype=np.int64)   # first lo column of slot j
    gcol_hi = np.zeros(NT, dtype=np.int64)
    for (j0, j1, gs, nlo, nhi) in batches:
        o = gs
        for j in range(j0, j1):
            gcol_lo[j] = o
            o += L[j]
        for j in range(j0, j1):
            gcol_hi[j] = o
            o += H[j]

    # per-core M tiles and gather indices
    m_tiles_all, idx_all = [], []
    for c in range(NC):
        m = np.zeros((NSUB, TILE, TILE), dtype=np.float32)
        idx = np.zeros(NSUB * TILE, dtype=np.int64)  # table idx per gather row
        sel_c = core_of == c
        for j in range(NT):
            sel = sel_c & (slot_of == j)
            for hi in (False, True):
                s = sel & (islo != hi)
                tc_, nl_, va_ = tcol[s], nloc[s], m2val[s]
                o = np.argsort(tc_, kind="stable")
                tc_, nl_, va_ = tc_[o], nl_[o], va_[o]
                base = gcol_hi[j] if hi else gcol_lo[j]
                nsub = H[j] if hi else L[j]
                e = np.arange(tc_.size)
                sub = base + e // TILE
                erow = e % TILE
                m[sub, erow, nl_] = va_
                gi = tc_ - (HALF if hi else 0)
                idx[base * TILE:(base + nsub) * TILE] = (HALF - HALF) if hi else 0
                idx[base * TILE + e] = gi
                # padding rows keep idx 0 (valid row, M row is zero)
        m_tiles_all.append(m.astype(bf16).reshape(NSUB * TILE, TILE))
        # wrap idx into [128, NSUB*8] int16: row i -> (partition i%16, col i//16),
        # replicated over the 8 partition groups
        iw = idx.reshape(NSUB, TILE // 16, 16).astype(np.int16)  # [sub, 8, 16]
        arr = np.zeros((128, NSUB * (TILE // 16)), dtype=np.int16)
        cols = iw.transpose(0, 1, 2).reshape(NSUB * (TILE // 16), 16)  # col-major over (sub, s)
        for rep in range(8):
            arr[rep * 16:(rep + 1) * 16, :] = cols.T
        idx_all.append(arr)

    # tables
    x = np.asarray(x, dtype=np.float32)
    x_table = np.zeros((NPAD, C), dtype=bf16)
    x_table[pos] = x.astype(bf16)

    meta = dict(L=L, H=H, T=T, NSUB=NSUB, batches=batches,
                gcol_lo=gcol_lo, gcol_hi=gcol_hi, pos=pos, pos_out=pos_out)
    return m_tiles_all, idx_all, x_table, meta


# --------------------------------------------------------------------------
# device program
# --------------------------------------------------------------------------

def _build(cfg, meta, mode="full"):
    import concourse.bacc as bacc
    import concourse.mybir as mybir
    import concourse.tile as tile
    from concourse.library_config import mlp
    do_gemm = mode in ("full", "nospmm", "noag")
    do_spmm = mode in ("full", "nogemm", "noag")

    C, K, NC, TILE = cfg["C"], cfg["K"], cfg["NC"], cfg["TILE"]
    NPAD, SHARD, TBMAX = cfg["NPAD"], cfg["SHARD"], cfg["TBMAX"]
    CH1 = cfg["CH1"]
    HALF = NC * CH1 * TILE
    NT = SHARD // TILE
    L, H, T = meta["L"], meta["H"], meta["T"]
    NSUB, batches = meta["NSUB"], meta["batches"]
    gcol_lo, gcol_hi = meta["gcol_lo"], meta["gcol_hi"]
    IDXW = TILE // 16

    nc = bacc.Bacc("TRN2", target_bir_lowering=False, debug=False,
                   num_devices=NC, num_swdge_queues=2,
                   dynamic_dma_scratch_size=32768)
    dt = mybir.dt
    x_table = nc.dram_tensor("x_table", [NPAD, C], dt.bfloat16, kind="ExternalInput")
    x_shard = nc.dram_tensor("x_shard", [SHARD, C], dt.bfloat16, kind="ExternalInput")
    m_in = nc.dram_tensor("m_tiles", [NSUB * TILE, TILE], dt.bfloat16, kind="ExternalInput")
    idx_in = nc.dram_tensor("idxs", [128, NSUB * IDXW], dt.int16, kind="ExternalInput")
    w_in = nc.dram_tensor("w_chunks", [2 * K * 128, C], dt.bfloat16, kind="ExternalInput")
    bias_in = nc.dram_tensor("bias_bcast", [128, C], dt.float32, kind="ExternalInput")
    negi_in = nc.dram_tensor("neg_id", [128, 128], dt.bfloat16, kind="ExternalInput")
    posi_in = nc.dram_tensor("pos_id", [128, 128], dt.bfloat16, kind="ExternalInput")
    out_t = nc.dram_tensor("out_shard", [SHARD, C], dt.float32, kind="ExternalOutput")
    R0 = CH1 * TILE               # chunk-0 rows per core (4096)
    CH = 10                       # per-batch staging cap (slots per batch)

    with tile.TileContext(nc) as tc:
        nc.gpsimd.load_library(mlp)
        with (
            tc.tile_pool(name="const", bufs=1) as const,
            tc.tile_pool(name="acc", bufs=NT) as accp,
            tc.tile_pool(name="g", bufs=3) as gp,
            tc.tile_pool(name="m", bufs=2) as mp,
            tc.tile_pool(name="tn", bufs=3) as tnp,
            tc.tile_pool(name="tp", bufs=3) as tpp,
            tc.tile_pool(name="tt", bufs=6) as ttp,
            tc.tile_pool(name="sp", bufs=3, space="PSUM") as spp,
            tc.tile_pool(name="gp", bufs=2, space="PSUM") as gpp,
            tc.tile_pool(name="tq", bufs=2, space="PSUM") as tqp,
            tc.tile_pool(name="dram", bufs=1, space="DRAM") as dram,
        ):
            idx_sb = const.tile([128, NSUB * IDXW], dt.int16)
            nc.sync.dma_start(idx_sb[:], idx_in[:])
            w_sb = const.tile([128, 2 * K, C], dt.bfloat16)
            nc.sync.dma_start(w_sb[:], w_in[:].rearrange("(w p) n -> p w n", p=128))
            bias_sb = const.tile([128, C], dt.float32)
            nc.sync.dma_start(bias_sb[:], bias_in[:])
            negi_sb = const.tile([128, 128], dt.bfloat16)
            nc.sync.dma_start(negi_sb[:], negi_in[:])
            posi_sb = const.tile([128, 128], dt.bfloat16)
            nc.sync.dma_start(posi_sb[:], posi_in[:])

            SRC = {}    # per-stage (chunk0, chunk1) gather source tensors
            B = {}      # per-core shard of T_k (AG input / local reload)
            B[0] = x_shard
            SRC[1] = (x_table[0:HALF, :], x_table[HALF:NPAD, :])
            for k in range(2, K):
                # per-chunk AllGather outputs (Shared; single writer each —
                # the checker rejects multi-writer Shared DRAM)
                SRC[k] = (dram.tile([NC * R0, C], dt.bfloat16,
                                    addr_space="Shared", name=f"sc_{k}_0")[:],
                          dram.tile([NC * (SHARD - R0), C], dt.bfloat16,
                                    addr_space="Shared", name=f"sc_{k}_1")[:])
            for k in range(1, K):
                B[k] = dram.tile([SHARD, C], dt.bfloat16, name=f"b_shard_{k}")

            acc = []
            for j in range(NT):
                a = accp.tile([128, C], dt.float32, tag="acc", name=f"acc_{j}")
                acc.append(a)

            def gemm(j, k, t_sb):
                """acc[j] (+)= T_k[tile j] @ W_k (+ bias at k==0).

                t_sb: SBUF [128, C] bf16 tile holding T_k rows. Transposed
                on TensorE (lhsT=t_sb chunk, rhs=I) so no DMA-transpose is
                needed — DMA-transpose overlapping a collective hangs the
                chip, which would serialize the chunked AllGathers."""
                if not do_gemm:
                    if k == 0:
                        nc.vector.tensor_copy(acc[j][:], bias_sb[:])
                    return
                gps = gpp.tile([128, C], dt.float32, tag="gps")
                for ch in range(2):
                    pT = tqp.tile([128, 128], dt.float32, tag="pt")
                    nc.tensor.matmul(pT[:], lhsT=t_sb[:, ch * 128:(ch + 1) * 128],
                                     rhs=posi_sb[:], start=True, stop=True)
                    tT = ttp.tile([128, 128], dt.bfloat16, tag="tt")
                    nc.scalar.activation(tT[:], pT[:],
                                         mybir.ActivationFunctionType.Copy)
                    nc.tensor.matmul(gps[:], lhsT=tT[:], rhs=w_sb[:, 2 * k + ch, :],
                                     start=(ch == 0), stop=(ch == 1))
                if k == 0:
                    nc.vector.tensor_add(acc[j][:], gps[:], bias_sb[:])
                else:
                    nc.vector.tensor_add(acc[j][:], acc[j][:], gps[:])

            # stage 0: out = x @ W0 + bias (x tiles loaded CH per DMA —
            # small transfers are fixed-cost dominated)
            for j0x in range(0, NT, CH):
                xt = tpp.tile([128, CH, C], dt.bfloat16, tag="tp")
                nc.sync.dma_start(
                    xt[:], x_shard[j0x * TILE:(j0x + CH) * TILE, :]
                    .rearrange("(t p) c -> p t c", p=128))
                for j in range(j0x, j0x + CH):
                    gemm(j, 0, xt[:, j - j0x, :])

            # stages 1..K-1; AllGathers fire per chunk (slots [0,CH1) then
            # the rest) so the collectives overlap the stage's own compute
            # and the next stage's chunk-0 gathers depend only on AG_0
            gq = [0]  # round-robin SWDGE queue cursor
            for k in range(1, K if do_spmm else 1):
                src_a, src_b = SRC[k]
                for (j0, j1, gstart, nlo, nhi) in batches:
                    nb = nlo + nhi
                    nj = j1 - j0
                    g = gp.tile([128, TBMAX, C], dt.bfloat16, tag="g")
                    m_b = mp.tile([128, TBMAX, TILE], dt.bfloat16, tag="m")
                    nc.scalar.dma_start(
                        m_b[:, 0:nb, :],
                        m_in[gstart * TILE:(gstart + nb) * TILE, :]
                        .rearrange("(s p) n -> p s n", p=128))
                    if k > 1:
                        tpb = tpp.tile([128, CH, C], dt.bfloat16, tag="tp")
                        nc.sync.dma_start(
                            tpb[:, 0:nj, :],
                            B[k - 2][j0 * TILE:j1 * TILE, :]
                            .rearrange("(t p) c -> p t c", p=128))
                    tb = tnp.tile([128, CH, C], dt.bfloat16, tag="tn")
                    # SWDGE descriptor ring holds ~1024 descriptors per queue;
                    # one gather call emits one descriptor per index, so cap
                    # calls at SUBCAP subtiles (SUBCAP*128 indices) and
                    # alternate queues so emission never waits on drain.
                    SUBCAP = 8
                    segs = ([(o, min(SUBCAP, nlo - o), src_a)
                             for o in range(0, nlo, SUBCAP)] +
                            [(nlo + o, min(SUBCAP, nhi - o), src_b)
                             for o in range(0, nhi, SUBCAP)])
                    for (o, n, s_ap) in segs:
                        nc.gpsimd.dma_gather(
                            g[:, o:o + n, :], s_ap,
                            idx_sb[:, (gstart + o) * IDXW:(gstart + o + n) * IDXW],
                            n * TILE, n * TILE, C, queue_num=gq[0])
                        gq[0] = (gq[0] + 1) % 2
                    for j in range(j0, j1):
                        psum = spp.tile([128, C], dt.float32, tag="sp")
                        subs = ([gcol_lo[j] + t for t in range(L[j])] +
                                [gcol_hi[j] + t for t in range(H[j])])
                        for ti, s in enumerate(subs):
                            nc.tensor.matmul(
                                psum[:], lhsT=m_b[:, s - gstart, :], rhs=g[:, s - gstart, :],
                                start=(ti == 0),
                                stop=(k == 1 and ti == len(subs) - 1))
                        if k > 1:
                            nc.tensor.matmul(psum[:], lhsT=negi_sb[:],
                                             rhs=tpb[:, j - j0, :],
                                             start=False, stop=True)
                        nc.scalar.activation(tb[:, j - j0, :], psum[:],
                                             mybir.ActivationFunctionType.Copy,
                                             scale=(0.5 if k == 1 else 1.0))
                        gemm(j, k, tb[:, j - j0, :])
                        if k == K - 1:
                            nc.sync.dma_start(
                                out_t[j * TILE:(j + 1) * TILE, :], acc[j][:])
                    if k < K - 1:
                        nc.sync.dma_start(
                            B[k][j0 * TILE:j1 * TILE, :]
                            .rearrange("(t p) c -> p t c", p=128),
                            tb[:, 0:nj, :])
                    # fire each chunk's AllGather as soon as its last slot's
                    # shard rows are written (batches never straddle chunks)
                    if k < K - 1 and mode != "noag" and j1 in (CH1, NT):
                        if j1 == CH1:
                            nc.gpsimd.collective_compute(
                                "AllGather", mybir.AluOpType.bypass,
                                replica_groups=[list(range(NC))],
                                ins=[B[k][0:R0, :].opt()],
                                outs=[SRC[k + 1][0].opt()])
                        else:
                            nc.gpsimd.collective_compute(
                                "AllGather", mybir.AluOpType.bypass,
                                replica_groups=[list(range(NC))],
                                ins=[B[k][R0:SHARD, :].opt()],
                                outs=[SRC[k + 1][1].opt()])

            if not do_spmm:
                for j in range(NT):
                    nc.sync.dma_start(out_t[j * TILE:(j + 1) * TILE, :], acc[j][:])
    nc.compile()
    return nc


# --------------------------------------------------------------------------
# entry point
# --------------------------------------------------------------------------

def kernel(x, edge_index, edge_weight, weight, bias):
    global LAST_EXEC_NS
    from concourse.bass_utils import run_bass_kernel_spmd

    cfg = CFG_FULL
    N, C, K, NC, SHARD = cfg["N"], cfg["C"], cfg["K"], cfg["NC"], cfg["SHARD"]
    x = np.asarray(x)
    weight = np.asarray(weight, dtype=np.float32)
    bias = np.asarray(bias, dtype=np.float32)

    m_tiles_all, idx_all, x_table, meta = _prep(x, edge_index, edge_weight, cfg)

    key = (tuple(meta["L"]), tuple(meta["H"]))
    if key not in _PROGRAM_CACHE:
        _PROGRAM_CACHE[key] = _build(cfg, meta)
    nc = _PROGRAM_CACHE[key]

    w_chunks = np.zeros((2 * K * 128, C), dtype=bf16)
    for k in range(K):
        for ch in range(2):
            w_chunks[(2 * k + ch) * 128:(2 * k + ch + 1) * 128] = \
                weight[k, ch * 128:(ch + 1) * 128, :].astype(bf16)
    bias_bcast = np.broadcast_to(bias, (128, C)).astype(np.float32).copy()
    neg_id = (-np.eye(128, dtype=np.float32)).astype(bf16)
    pos_id = np.eye(128, dtype=np.float32).astype(bf16)

    # core-local-order copy of x (x_table is chunk-major, so its SHARD
    # slices no longer correspond to cores)
    pos_out = meta["pos_out"]
    x_core = np.zeros((cfg["NPAD"], C), dtype=bf16)
    x_core[pos_out] = x.astype(bf16)

    in_maps = []
    for c in range(NC):
        in_maps.append({
            "x_table": x_table,
            "x_shard": x_core[c * SHARD:(c + 1) * SHARD],
            "m_tiles": m_tiles_all[c],
            "idxs": idx_all[c],
            "w_chunks": w_chunks,
            "bias_bcast": bias_bcast,
            "neg_id": neg_id,
            "pos_id": pos_id,
        })

    trace = bool(os.environ.get("CHEB_TRACE"))
    kw = {}
    if trace:
        kw = dict(trace=True, tmpdir=os.environ.get("CHEB_TRACE_DIR") or None)
    res = run_bass_kernel_spmd(nc, in_maps, core_ids=list(range(NC)), **kw)
    LAST_EXEC_NS = res.exec_time_ns

    shards = [res.results[c]["out_shard"] for c in range(NC)]
    full = np.concatenate(shards, axis=0)      # [NPAD(core-local order), C]
    out = full[pos_out]                        # back to node order
    return np.ascontiguousarray(out.astype(np.float32))

